# revision 9
# baseline (speedup 1.0000x reference)
"""DepthAwareConv2d Trainium2 kernel (bf16 v2).

Math: the reference's depth-modulated im2col GEMM is exactly
    out = conv2d(x * depth, weight, stride=1, pad=1) + bias
(depth broadcasts over channels; unfold(x)*unfold(depth) = unfold(x*depth)).

Sharding (8 cores): data-parallel over N (4 images) x spatial-parallel over
image row halves.  Core cid handles n = cid//2, row half = cid%2 (output rows
[0,64) or [64,128)), computing all 256 output channels for its half.  The
host ships each core its 64 input rows plus one halo/zero row on each side
(66 rows total), so the device program is identical on every core (SPMD) and
no collectives are needed.

v2 vs baseline (94.1us):
  * bf16 everywhere on the PE path.  bf16 matmuls lower to LDWEIGHTS+MATMUL
    pairs whose weight loads the PE reorder window hoists into the background
    weight buffer (hidden under the previous matmul) -- fp32r self-loading
    matmuls couldn't, costing ~47ns x 288 of PE bubbles.
  * weights DMA directly as bf16 (host-cast): kills the DMA->ScalarE fp32r
    round-copy chain that gated the first real matmul at 18.5us.
  * inputs ride the Sync-engine HWDGE, outputs the ScalarE HWDGE, so output
    blocks never queue behind input chunks.
  * bf16 in/out halves all DMA traffic (in 4.9MB, out 4.2MB per core); host
    upcasts the output to fp32.

Per-core device kernel:
  1. DMA 66 x-rows + partition-broadcast depth rows (both bf16) into SBUF
     chunks; DVE-multiply into column-padded bf16 ypad (128 part, 66 x 130).
  2. Shift-conv: per 4-row output block and 128-wide out-channel block,
     9 accumulating bf16 matmuls (stationary = 128x128 weight tap, moving =
     shifted 4x128 window, free dim 512 = one PSUM bank) into one PSUM bank.
  3. ScalarE Identity(+bias) PSUM->SBUF (bf16), DMA out.
"""

import ml_dtypes
import numpy as np

import concourse.bass as bass
import concourse.mybir as mybir
import concourse.tile as tile
from concourse import bacc
from concourse.bass_utils import run_bass_kernel_spmd

N, C, O, H, W = 4, 128, 256, 128, 128
HSH = H // 2  # output rows per core
HIN = HSH + 2  # input rows per core incl. halo/zero rows
NCORES = 8
F32 = mybir.dt.float32
BF16 = mybir.dt.bfloat16
ACT_IDENT = mybir.ActivationFunctionType.Identity
NPBF16 = ml_dtypes.bfloat16

RB = 4  # output rows per matmul tile (free dim RB*W = 512, one PSUM bank)
# image rows per load/multiply chunk; small first chunks let the first
# matmul block (which only needs rows 0..5) start as early as possible
CHUNKS = (4, 4, 8, 12, 12, 12, 14)  # sums to HIN = 66; boundaries all 0 mod 4
NWARM = 6  # zero matmuls bridging the input-DMA window (HAM warm + no gap)

_CACHE = {}


def build_nc():
    nc = bacc.Bacc("TRN2", target_bir_lowering=False, debug=False, num_devices=NCORES)
    xs = nc.declare_dram_parameter("xs", [C, HIN, W], BF16, isOutput=False)
    dep = nc.declare_dram_parameter("dep", [HIN * W], BF16, isOutput=False)
    wt = nc.declare_dram_parameter("wt", [C, 2, 9, O // 2], BF16, isOutput=False)
    bb = nc.declare_dram_parameter("bb", [O // 2, 2], F32, isOutput=False)
    out = nc.declare_dram_parameter("out", [O, HSH, W], BF16, isOutput=True)

    with tile.TileContext(nc) as tc:
        with (
            tc.tile_pool(name="big", bufs=1) as big,
            tc.tile_pool(name="wp", bufs=1) as wp,
            tc.tile_pool(name="ch", bufs=3) as chp,
            tc.tile_pool(name="op", bufs=4) as op,
            tc.tile_pool(name="pp", bufs=8, space="PSUM") as pp,
        ):
            ypad = big.tile([C, HIN, W + 2], BF16)
            wsb = wp.tile([C, 2, 9, O // 2], BF16)
            bsb = wp.tile([O // 2, 2], F32)  # bsb[p, ob] = bias[ob*128 + p]
            ztile = wp.tile([C, RB * W], BF16)

            # zeros: warmup operands + ypad column borders
            nc.vector.memset(ztile, 0.0)
            nc.vector.memset(ypad[:, :, 0], 0.0)
            nc.vector.memset(ypad[:, :, W + 1], 0.0)

            # PE warm-up: zero bf16 matmuls bridge the input-DMA window so the
            # HAM clock gate reaches 8/8 before the real train starts and the
            # PE never idles >3.4us.
            warm = pp.tile([O // 2, RB, W], F32, tag="ps")
            for _ in range(NWARM):
                nc.tensor.matmul(warm, ztile[:, :128], ztile, start=True, stop=True)

            CMAX = max(CHUNKS)
            bases = []
            b = 0
            for ch in CHUNKS:
                bases.append(b)
                b += ch
            tiles = {}

            def chunk_dma(ci):
                # x rides the Sync HWDGE, depth the ScalarE HWDGE: the two
                # halves of each chunk transfer in parallel instead of
                # serializing on one ring.
                r0, ch = bases[ci], CHUNKS[ci]
                xb = chp.tile([C, CMAX, W], BF16, tag="xb", name=f"xb{ci}")
                db = chp.tile([C, CMAX, W], BF16, tag="db", name=f"db{ci}")
                nc.sync.dma_start(out=xb[:, :ch], in_=xs[:, r0 : r0 + ch, :])
                nc.scalar.dma_start(
                    out=db[:, :ch],
                    in_=dep.ap()[r0 * W : (r0 + ch) * W].partition_broadcast(C),
                )
                tiles[ci] = (xb, db)

            def mul_rows(r0, r1):
                # 4-aligned multiply blocks: conv block t reads ypad rows
                # 4t..4t+5; keeping writer granularity 4-aligned means Tile's
                # (quantized) range-overlap check never drags in a writer one
                # byte past the true read range.
                ci = next(
                    i for i, base in enumerate(bases) if base <= r0 < base + CHUNKS[i]
                )
                xb, db = tiles[ci]
                lo = r0 - bases[ci]
                nc.vector.tensor_mul(
                    out=ypad[:, r0:r1, 1 : W + 1],
                    in0=xb[:, lo : lo + (r1 - r0)],
                    in1=db[:, lo : lo + (r1 - r0)],
                )

            # input DMA order = consumption order.  The ScalarE queue (pure
            # DMA issuing now -- no act-table load) starts earliest, so depth
            # chunks + the ob=0 weight half ride it; x chunks ride Sync.
            chunk_dma(0)
            chunk_dma(1)
            nc.scalar.dma_start(out=wsb[:, 0], in_=wt.ap()[:, 0])
            nc.sync.dma_start(out=wsb[:, 1], in_=wt.ap()[:, 1])
            nc.scalar.dma_start(out=bsb, in_=bb.ap())
            mul_rows(0, 4)
            mul_rows(4, 8)
            for ci in range(2, len(CHUNKS)):
                chunk_dma(ci)
            r = CHUNKS[0]
            while r < HIN:
                r1 = min(r + 4, HIN)
                mul_rows(r, r1)
                r = r1

            for rb in range(0, HSH, RB):
                for ob in range(2):
                    ps = pp.tile([O // 2, RB, W], F32, tag="ps", name=f"ps{rb}_{ob}")
                    for p in range(9):
                        i, j = divmod(p, 3)
                        nc.tensor.matmul(
                            ps,
                            wsb[:, ob, p],
                            ypad[:, rb + i : rb + i + RB, j : j + W],
                            start=(p == 0),
                            stop=(p == 8),
                        )
                    osb = op.tile(
                        [O // 2, RB, W], BF16, tag="osb", name=f"osb{rb}_{ob}"
                    )
                    # PSUM drain on DVE (not ScalarE): avoids the 1.3us
                    # act-table load that would otherwise delay ScalarE's
                    # first input-DMA issue at kernel start
                    nc.vector.tensor_scalar_add(
                        out=osb, in0=ps, scalar1=bsb[:, ob : ob + 1]
                    )
                    # output DMA on the Sync HWDGE, independent of the
                    # input stream on ScalarE
                    nc.sync.dma_start(
                        out=out[ob * 128 : (ob + 1) * 128, rb : rb + RB, :],
                        in_=osb,
                    )

    nc.compile()
    return nc


def _get_nc():
    if "nc" not in _CACHE:
        _CACHE["nc"] = build_nc()
    return _CACHE["nc"]


def make_in_maps(x, depth, weight, bias):
    x = np.asarray(x, np.float32)
    depth = np.asarray(depth, np.float32)
    weight = np.asarray(weight, np.float32)
    bias = np.asarray(bias, np.float32)
    # (O, C, 3, 3) -> (C, ob, tap=i*3+j, o) with o = local index in the
    # 128-wide out-channel half ob
    wt9 = np.ascontiguousarray(
        np.transpose(
            weight.reshape(2, O // 2, C, 3, 3), (2, 0, 3, 4, 1)
        ).reshape(C, 2, 9, O // 2)
    ).astype(NPBF16)
    bb = np.ascontiguousarray(bias.reshape(2, O // 2).T)
    xb = x.astype(NPBF16)
    db = depth.astype(NPBF16)
    in_maps = []
    for cid in range(NCORES):
        n, hh = divmod(cid, 2)
        xsh = np.zeros((C, HIN, W), NPBF16)
        dsh = np.zeros((HIN, W), NPBF16)
        if hh == 0:
            xsh[:, 1:] = xb[n, :, : HSH + 1]
            dsh[1:] = db[n, 0, : HSH + 1]
        else:
            xsh[:, :-1] = xb[n, :, HSH - 1 :]
            dsh[:-1] = db[n, 0, HSH - 1 :]
        in_maps.append(
            {
                "xs": xsh,
                "dep": np.ascontiguousarray(dsh.reshape(-1)),
                "wt": wt9,
                "bb": bb,
            }
        )
    return in_maps


def gather_out(results):
    out = np.empty((N, O, H, W), np.float32)
    for cid in range(NCORES):
        n, hh = divmod(cid, 2)
        out[n, :, hh * HSH : (hh + 1) * HSH] = results[cid]["out"].astype(np.float32)
    return out


def kernel(x, depth, camera_params, weight, bias):
    nc = _get_nc()
    in_maps = make_in_maps(x, depth, weight, bias)
    res = run_bass_kernel_spmd(nc, in_maps, list(range(NCORES)))
    return gather_out(res.results)


# revision 14
# speedup vs baseline: 1.2110x; 1.2110x over previous
"""DepthAwareConv2d Trainium2 kernel (bf16 v2).

Math: the reference's depth-modulated im2col GEMM is exactly
    out = conv2d(x * depth, weight, stride=1, pad=1) + bias
(depth broadcasts over channels; unfold(x)*unfold(depth) = unfold(x*depth)).

Sharding (8 cores): data-parallel over N (4 images) x spatial-parallel over
image row halves.  Core cid handles n = cid//2, row half = cid%2 (output rows
[0,64) or [64,128)), computing all 256 output channels for its half.  The
host ships each core its 64 input rows plus one halo/zero row on each side
(66 rows total), so the device program is identical on every core (SPMD) and
no collectives are needed.

v2 vs baseline (94.1us):
  * bf16 everywhere on the PE path.  bf16 matmuls lower to LDWEIGHTS+MATMUL
    pairs whose weight loads the PE reorder window hoists into the background
    weight buffer (hidden under the previous matmul) -- fp32r self-loading
    matmuls couldn't, costing ~47ns x 288 of PE bubbles.
  * weights DMA directly as bf16 (host-cast): kills the DMA->ScalarE fp32r
    round-copy chain that gated the first real matmul at 18.5us.
  * inputs ride the Sync-engine HWDGE, outputs the ScalarE HWDGE, so output
    blocks never queue behind input chunks.
  * bf16 in/out halves all DMA traffic (in 4.9MB, out 4.2MB per core); host
    upcasts the output to fp32.

Per-core device kernel:
  1. DMA 66 x-rows + partition-broadcast depth rows (both bf16) into SBUF
     chunks; DVE-multiply into column-padded bf16 ypad (128 part, 66 x 130).
  2. Shift-conv: per 4-row output block and 128-wide out-channel block,
     9 accumulating bf16 matmuls (stationary = 128x128 weight tap, moving =
     shifted 4x128 window, free dim 512 = one PSUM bank) into one PSUM bank.
  3. ScalarE Identity(+bias) PSUM->SBUF (bf16), DMA out.
"""

import ml_dtypes
import numpy as np

import concourse.bass as bass
import concourse.mybir as mybir
import concourse.tile as tile
from concourse import bacc
from concourse.bass_utils import run_bass_kernel_spmd

N, C, O, H, W = 4, 128, 256, 128, 128
HSH = H // 2  # output rows per core
HIN = HSH + 2  # input rows per core incl. halo/zero rows
NCORES = 8
F32 = mybir.dt.float32
BF16 = mybir.dt.bfloat16
ACT_IDENT = mybir.ActivationFunctionType.Identity
NPBF16 = ml_dtypes.bfloat16

RB = 4  # output rows per matmul tile (free dim RB*W = 512, one PSUM bank)
# image rows per load/multiply chunk; small first chunks let the first
# matmul block (which only needs rows 0..5) start as early as possible
CHUNKS = (4, 4, 8, 12, 12, 12, 14)  # sums to HIN = 66; boundaries all 0 mod 4
NWARM = 8  # zero matmuls bridging the input-DMA window (HAM warm + no gap)

_CACHE = {}


def build_nc():
    nc = bacc.Bacc("TRN2", target_bir_lowering=False, debug=False, num_devices=NCORES)
    xs = nc.declare_dram_parameter("xs", [C, HIN, W], BF16, isOutput=False)
    dep = nc.declare_dram_parameter("dep", [HIN * W], BF16, isOutput=False)
    wt = nc.declare_dram_parameter("wt", [C, 2, 9, O // 2], BF16, isOutput=False)
    bb = nc.declare_dram_parameter("bb", [O // 2, 2], F32, isOutput=False)
    out = nc.declare_dram_parameter("out", [O, HSH, W], BF16, isOutput=True)

    with tile.TileContext(nc) as tc:
        with (
            tc.tile_pool(name="big", bufs=1) as big,
            tc.tile_pool(name="wp", bufs=1) as wp,
            tc.tile_pool(name="ch", bufs=3) as chp,
            tc.tile_pool(name="op", bufs=4) as op,
            tc.tile_pool(name="pp", bufs=8, space="PSUM") as pp,
        ):
            ypad = big.tile([C, HIN, W + 2], BF16)
            wsb = wp.tile([C, 2, 9, O // 2], BF16)
            bsb = wp.tile([O // 2, 2], F32)  # bsb[p, ob] = bias[ob*128 + p]
            ztile = wp.tile([C, RB * W], BF16)

            # zeros: warmup operands + ypad column borders
            nc.vector.memset(ztile, 0.0)
            nc.vector.memset(ypad[:, :, 0], 0.0)
            nc.vector.memset(ypad[:, :, W + 1], 0.0)

            # PE warm-up: zero bf16 matmuls bridge the input-DMA window so the
            # HAM clock gate reaches 8/8 before the real train starts and the
            # PE never idles >3.4us.
            warm = pp.tile([O // 2, RB, W], F32, tag="ps")
            for _ in range(NWARM):
                nc.tensor.matmul(warm, ztile[:, :128], ztile, start=True, stop=True)

            CMAX = max(CHUNKS)
            bases = []
            b = 0
            for ch in CHUNKS:
                bases.append(b)
                b += ch
            tiles = {}

            def chunk_dma(ci, xeng=None, deng=None):
                # default split: x rides the Sync HWDGE, depth the ScalarE
                # HWDGE, so the two halves of a chunk transfer in parallel
                # instead of serializing on one ring.
                r0, ch = bases[ci], CHUNKS[ci]
                xb = chp.tile([C, CMAX, W], BF16, tag="xb", name=f"xb{ci}")
                db = chp.tile([C, CMAX, W], BF16, tag="db", name=f"db{ci}")
                (xeng or nc.sync).dma_start(out=xb[:, :ch], in_=xs[:, r0 : r0 + ch, :])
                (deng or nc.scalar).dma_start(
                    out=db[:, :ch],
                    in_=dep.ap()[r0 * W : (r0 + ch) * W].partition_broadcast(C),
                )
                tiles[ci] = (xb, db)

            def mul_rows(r0, r1):
                # 4-aligned multiply blocks: conv block t reads ypad rows
                # 4t..4t+5; keeping writer granularity 4-aligned means Tile's
                # (quantized) range-overlap check never drags in a writer one
                # byte past the true read range.
                ci = next(
                    i for i, base in enumerate(bases) if base <= r0 < base + CHUNKS[i]
                )
                xb, db = tiles[ci]
                lo = r0 - bases[ci]
                nc.vector.tensor_mul(
                    out=ypad[:, r0:r1, 1 : W + 1],
                    in0=xb[:, lo : lo + (r1 - r0)],
                    in1=db[:, lo : lo + (r1 - r0)],
                )

            # input DMA order = consumption order.  ScalarE's queue is held
            # up ~1.3us by the act-table load, so the critical chunk0 pair +
            # first weight half ride Sync; chunk1 rides ScalarE (ready just
            # after the table load), later chunks split x/Sync d/ScalarE.
            chunk_dma(0, xeng=nc.sync, deng=nc.sync)
            nc.sync.dma_start(out=wsb[:, 0], in_=wt.ap()[:, 0])
            chunk_dma(1, xeng=nc.scalar, deng=nc.scalar)
            nc.sync.dma_start(out=wsb[:, 1], in_=wt.ap()[:, 1])
            nc.scalar.dma_start(out=bsb, in_=bb.ap())
            mul_rows(0, 4)
            mul_rows(4, 8)
            for ci in range(2, len(CHUNKS)):
                chunk_dma(ci)
            r = CHUNKS[0]
            while r < HIN:
                r1 = min(r + 4, HIN)
                mul_rows(r, r1)
                r = r1

            def conv_block(r0, nrows, ob, name):
                # one accumulation group: rows [r0, r0+nrows), out-half ob
                ps = pp.tile([O // 2, nrows, W], F32, tag="ps", name=f"ps{name}")
                for p in range(9):
                    i, j = divmod(p, 3)
                    nc.tensor.matmul(
                        ps,
                        wsb[:, ob, p],
                        ypad[:, r0 + i : r0 + i + nrows, j : j + W],
                        start=(p == 0),
                        stop=(p == 8),
                    )
                osb = op.tile([O // 2, nrows, W], BF16, tag="osb", name=f"osb{name}")
                nc.scalar.activation(
                    out=osb,
                    in_=ps,
                    func=ACT_IDENT,
                    bias=bsb[:, ob : ob + 1],
                    scale=1.0,
                )
                # output DMA on the ScalarE HWDGE: rides the engine that
                # produced the data, independent of the input stream
                nc.scalar.dma_start(
                    out=out[ob * 128 : (ob + 1) * 128, r0 : r0 + nrows, :],
                    in_=osb,
                )

            for rb in range(0, HSH, RB):
                for ob in range(2):
                    if rb == HSH - RB and ob == 1:
                        # run the very last block as two 2-row groups so its
                        # final drain + output DMA start ~0.9us earlier,
                        # shortening the kernel tail
                        conv_block(rb, 2, 1, f"{rb}_{ob}a")
                        conv_block(rb + 2, 2, 1, f"{rb}_{ob}b")
                    else:
                        conv_block(rb, RB, ob, f"{rb}_{ob}")

    nc.compile()
    return nc


def _get_nc():
    if "nc" not in _CACHE:
        _CACHE["nc"] = build_nc()
    return _CACHE["nc"]


def make_in_maps(x, depth, weight, bias):
    x = np.asarray(x, np.float32)
    depth = np.asarray(depth, np.float32)
    weight = np.asarray(weight, np.float32)
    bias = np.asarray(bias, np.float32)
    # (O, C, 3, 3) -> (C, ob, tap=i*3+j, o) with o = local index in the
    # 128-wide out-channel half ob
    wt9 = np.ascontiguousarray(
        np.transpose(
            weight.reshape(2, O // 2, C, 3, 3), (2, 0, 3, 4, 1)
        ).reshape(C, 2, 9, O // 2)
    ).astype(NPBF16)
    bb = np.ascontiguousarray(bias.reshape(2, O // 2).T)
    xb = x.astype(NPBF16)
    db = depth.astype(NPBF16)
    in_maps = []
    for cid in range(NCORES):
        n, hh = divmod(cid, 2)
        xsh = np.zeros((C, HIN, W), NPBF16)
        dsh = np.zeros((HIN, W), NPBF16)
        if hh == 0:
            xsh[:, 1:] = xb[n, :, : HSH + 1]
            dsh[1:] = db[n, 0, : HSH + 1]
        else:
            xsh[:, :-1] = xb[n, :, HSH - 1 :]
            dsh[:-1] = db[n, 0, HSH - 1 :]
        in_maps.append(
            {
                "xs": xsh,
                "dep": np.ascontiguousarray(dsh.reshape(-1)),
                "wt": wt9,
                "bb": bb,
            }
        )
    return in_maps


def gather_out(results):
    out = np.empty((N, O, H, W), np.float32)
    for cid in range(NCORES):
        n, hh = divmod(cid, 2)
        out[n, :, hh * HSH : (hh + 1) * HSH] = results[cid]["out"].astype(np.float32)
    return out


def kernel(x, depth, camera_params, weight, bias):
    nc = _get_nc()
    in_maps = make_in_maps(x, depth, weight, bias)
    res = run_bass_kernel_spmd(nc, in_maps, list(range(NCORES)))
    return gather_out(res.results)
